# revision 1
# baseline (speedup 1.0000x reference)
"""Trainium2 Bass kernel for nn_Attention_82403242541756.

Reference semantics (with the dim-0 chunk bug):
  qkv = inputs @ W_qkv + b_qkv                  # [3, 2048, 3072]
  q, k, v = split(qkv, 3, axis=0)               # batch split! q=batch0, k=batch1, v=batch2
  each chunk [1, 2048, 3072] flat-reinterpreted to (3, 16, 2048, 64) = 48 "heads"
  scoresT softmax (no max needed; |scores| < 2.2), ctx, flat-reinterpret, @ W_out + b_out

Sharding (zero communication): core c takes seq rows [256c, 256c+256) of all 3
batch items. Head g's flat chunk [g*131072, (g+1)*131072) of a batch's [2048*3072]
QKV output aligns exactly with rows [256c, 256c+256) for g in [6c, 6c+6), and the
output-side reinterpret puts head g at rows [128g, 128g+128) of the flattened
[6144, 1024] context, i.e. rows [768c, 768c+768) of the final output per core.
"""

import sys

sys.path.insert(0, "/opt/trn_rl_repo")

import numpy as np
import ml_dtypes

from concourse import bacc, bass, mybir, tile
from concourse.bass_utils import run_bass_kernel_spmd

BF16 = mybir.dt.bfloat16
F32 = mybir.dt.float32
AF = mybir.ActivationFunctionType
ALU = mybir.AluOpType

P = 128
N_CORES = 8
SEQ = 2048
H = 1024
HEADS_PER_CORE = 6
ROWS = 256  # seq rows per core
SCALE = float(H) ** -0.5  # 1/32, folded into the exp activation

_NC_CACHE = {}


def _build():
    nc = bacc.Bacc()

    xt_e = nc.declare_dram_parameter("xt", [P, 8, 768], BF16, isOutput=False)
    wq_e = nc.declare_dram_parameter("wq", [P, 8, 3072], BF16, isOutput=False)
    bq_e = nc.declare_dram_parameter("bq", [P, 3072], F32, isOutput=False)
    wo_e = nc.declare_dram_parameter("wo", [64, 16, 1024], BF16, isOutput=False)
    bo_e = nc.declare_dram_parameter("bo", [P, 8], F32, isOutput=False)
    out_e = nc.declare_dram_parameter("outt", [1024, 768], F32, isOutput=True)

    with tile.TileContext(nc) as tc:
        with (
            tc.tile_pool(name="dram", bufs=1, space="DRAM") as dp,
            tc.tile_pool(name="qk", bufs=4) as qkp,
            tc.tile_pool(name="vex", bufs=2) as vxp,
            tc.tile_pool(name="scps", bufs=2, space="PSUM") as scps_p,
            tc.tile_pool(name="expp", bufs=2) as expp,
        ):
            # Padded to 128 cols so the bf16 XBAR DMA-transpose readback is legal.
            yq = dp.tile([12288, 128], BF16)
            yk = dp.tile([12288, 128], BF16)
            yv = dp.tile([12288, 64], BF16)
            yq_v = yq.rearrange("(r j) d -> r j d", j=48)
            yk_v = yk.rearrange("(r j) d -> r j d", j=48)
            yv_v = yv.rearrange("(r j) d -> r (j d)", j=48)

            # ---------------- Phase 1 pools (closed mid-stream, after the
            # b=2 block, so attention SBUF/PSUM can reuse their space) -------
            import contextlib

            es = contextlib.ExitStack()
            w1p = es.enter_context(tc.tile_pool(name="w1", bufs=1))
            ps1 = es.enter_context(tc.tile_pool(name="ps1", bufs=4, space="PSUM"))
            ybp = es.enter_context(tc.tile_pool(name="yb", bufs=4))

            xt_sb = w1p.tile([P, 8, 768], BF16)
            nc.scalar.dma_start(xt_sb[:], xt_e[:])
            wq_sb = w1p.tile([P, 8, 3072], BF16)
            # stream W_qkv in consumption order: first the 0:1536 column half
            # of every k-tile (what the first QKV half-pass reads), then the
            # rest. Alternate SP/ACT issue queues for 2x stream bandwidth
            # (ACT's queue is empty this early).
            for half in range(2):
                for k in range(8):
                    eng = nc.sync if k % 2 == 0 else nc.scalar
                    eng.dma_start(
                        wq_sb[:, k, 1536 * half : 1536 * (half + 1)],
                        wq_e[:, k, 1536 * half : 1536 * (half + 1)],
                    )
            bq_sb = w1p.tile([P, 3072], F32)
            nc.sync.dma_start(bq_sb[:], bq_e[:])

            def emit_qkv_block(b):
                for m in range(2):
                    psums = {}
                    for half in range(2):
                        for nb3 in range(3):
                            psums[3 * half + nb3] = ps1.tile(
                                [P, 512], F32, name=f"yps{3*half+nb3}", tag="yps"
                            )
                        for k in range(8):
                            lhs = xt_sb[:, k, b * 256 + 128 * m : b * 256 + 128 * (m + 1)]
                            for nb3 in range(3):
                                nb = 3 * half + nb3
                                nc.tensor.matmul(
                                    psums[nb][:],
                                    lhsT=lhs,
                                    rhs=wq_sb[:, k, 512 * nb : 512 * (nb + 1)],
                                    start=(k == 0),
                                    stop=(k == 7),
                                )
                    for nb in range(6):
                        if b < 2:
                            # widened [*, *, 128] with zeroed pad cols 64:128 so
                            # the DMA-transpose readback sees defined data
                            ybuf = ybp.tile([P, 8, 128], BF16, tag="ybw")
                            nc.vector.memset(ybuf[:, :, 64:128], 0.0)
                            nc.vector.tensor_tensor(
                                ybuf[:, :, 0:64],
                                psums[nb].rearrange("p (j d) -> p j d", d=64),
                                bq_sb[:, 512 * nb : 512 * (nb + 1)].rearrange(
                                    "p (j d) -> p j d", d=64
                                ),
                                ALU.add,
                            )
                            dst = (yq_v if b == 0 else yk_v)[
                                128 * m : 128 * (m + 1), 8 * nb : 8 * (nb + 1), :
                            ]
                            nc.sync.dma_start(dst, ybuf[:])
                        else:
                            ybuf = ybp.tile([P, 512], BF16, tag="ybn")
                            nc.vector.tensor_tensor(
                                ybuf[:],
                                psums[nb][:],
                                bq_sb[:, 512 * nb : 512 * (nb + 1)],
                                ALU.add,
                            )
                            nc.sync.dma_start(
                                yv_v[128 * m : 128 * (m + 1), 512 * nb : 512 * (nb + 1)],
                                ybuf[:],
                            )

            def emit_vx(l):
                # must be emitted AFTER the b=2 qkv block: Tile orders by
                # emission history, a read emitted before the write would
                # see stale data
                vx = vxp.tile([P, 16, 65], BF16, name=f"vx{l}", tag="vx")
                nc.vector.memset(vx[:, :, 64:65], 1.0)
                nc.sync.dma_start(
                    vx[:, :, 0:64],
                    yv[SEQ * l : SEQ * (l + 1), :].rearrange("(so p) d -> p so d", p=P),
                )
                return vx

            def emit_head_frontend(l):
                # head 0's transposes ride the (still-empty) ACT queue so they
                # skip the SP backlog; later heads stay on SP since the ACT
                # instruction stream is then full of exps
                dma_eng = nc.scalar if l == 0 else nc.sync
                qT = qkp.tile([P, SEQ], BF16, tag="qk", name=f"qT{l}")
                dma_eng.dma_start(qT[:], yq[SEQ * l : SEQ * (l + 1), :], transpose=True)
                kT = qkp.tile([P, SEQ], BF16, tag="qk", name=f"kT{l}")
                dma_eng.dma_start(kT[:], yk[SEQ * l : SEQ * (l + 1), :], transpose=True)
                expTs = []
                for th in range(2):
                    expT = expp.tile([P, 8, SEQ], BF16, tag="expT", name=f"expT{l}_{th}")
                    expTs.append(expT)
                    for t8 in range(8):
                        tt = 8 * th + t8
                        for hh in range(2):
                            sc = scps_p.tile([P, 1024], F32, name=f"sc{l}_{tt}_{hh}", tag="sc")
                            for s2 in range(2):
                                s0 = 1024 * hh + 512 * s2
                                nc.tensor.matmul(
                                    sc[:, 512 * s2 : 512 * (s2 + 1)],
                                    lhsT=kT[0:64, 128 * tt : 128 * (tt + 1)],
                                    rhs=qT[0:64, s0 : s0 + 512],
                                    start=True,
                                    stop=True,
                                )
                            nc.scalar.activation(
                                expT[:, t8, 1024 * hh : 1024 * (hh + 1)],
                                sc[:],
                                AF.Exp,
                                scale=SCALE,
                            )
                return expTs

            emit_qkv_block(0)
            emit_qkv_block(1)
            # Head-0 scores/exp emitted BEFORE the b=2 block so the scheduler
            # starts ACT as soon as yq/yk land; b=2 then fills PE slack.
            h0_expTs = emit_head_frontend(0)
            emit_qkv_block(2)
            es.close()  # release w1/ps1/yb space for the attention phase

            # ---------------- Phase 2: attention + out-proj ----------------
            with (
                tc.tile_pool(name="w2", bufs=1) as w2p,
                tc.tile_pool(name="rs", bufs=2) as rsp,
                tc.tile_pool(name="rbc", bufs=2) as rbcp,
                tc.tile_pool(name="stg", bufs=2) as stgp,
            ):
                wo_sb = w2p.tile([64, 16, 1024], BF16)
                nc.sync.dma_start(wo_sb[:], wo_e[:])
                bo_sb = w2p.tile([P, 8], F32)
                nc.sync.dma_start(bo_sb[:], bo_e[:])
                # merged normalized transposed context [d, s_parity, 6*128 rows]
                ctxn_all = w2p.tile([64, 16, 768], BF16)

                ctx_es = contextlib.ExitStack()
                ctxps_p = ctx_es.enter_context(
                    tc.tile_pool(name="ctxps", bufs=1, space="PSUM")
                )

                def emit_head_backend(l, vx, expTs):
                    ctxps = ctxps_p.tile([65, SEQ], F32, name=f"ctxps{l}", tag="ctxps")
                    for th in range(2):
                        for t8 in range(8):
                            tt = 8 * th + t8
                            for ss in range(4):
                                nc.tensor.matmul(
                                    ctxps[:, 512 * ss : 512 * (ss + 1)],
                                    lhsT=vx[:, tt, :],
                                    rhs=expTs[th][:, t8, 512 * ss : 512 * (ss + 1)],
                                    start=(tt == 0),
                                    stop=(tt == 15),
                                )
                    emit_norm(l, ctxps)

                def emit_norm(l, ctxps):
                    # Copy ctx psum -> sbuf f32 immediately so the psum tile
                    # frees for the next head; the rest of the normalize chain
                    # runs off the critical path.
                    ctxf = rsp.tile([65, SEQ], F32, tag="ctxf")
                    nc.vector.tensor_copy(out=ctxf[:], in_=ctxps[:])
                    rs = rsp.tile([P, 16], F32, tag="rs")
                    nc.sync.dma_start(rs[:], ctxf[64:65, :])
                    rr = rsp.tile([P, 16], F32, tag="rr")
                    nc.vector.reciprocal(rr[:], rs[:])
                    rrow_d = dp.tile([1, SEQ], F32, name=f"rrowd{l}", tag="rrowd")
                    nc.sync.dma_start(rrow_d[:], rr[:])
                    rbc = rbcp.tile([64, SEQ], F32)
                    nc.gpsimd.dma_start(
                        rbc[:], rrow_d[0:1, :].to_broadcast([64, SEQ])
                    )
                    # normalize into the merged layout [d, sp, 128l + r]
                    nc.vector.tensor_tensor(
                        ctxn_all[:, :, 128 * l : 128 * (l + 1)],
                        ctxf[0:64, :].rearrange("d (r t) -> d t r", t=16),
                        rbc.rearrange("d (r t) -> d t r", t=16),
                        ALU.mult,
                    )

                def emit_outproj_cols(c0, c1, pool, tag):
                    n = c1 - c0
                    for m in range(8):
                        ops = pool.tile([P, n], F32, name=f"op_{tag}_{m}", tag=tag)
                        for sp in range(16):
                            nc.tensor.matmul(
                                ops[:],
                                lhsT=wo_sb[:, sp, 128 * m : 128 * (m + 1)],
                                rhs=ctxn_all[:, sp, c0:c1],
                                start=(sp == 0),
                                stop=(sp == 15),
                            )
                        stg = stgp.tile([P, n], F32, tag="stg")
                        nc.vector.tensor_scalar(
                            stg[:], ops[:], bo_sb[:, m : m + 1], None, ALU.add
                        )
                        nc.sync.dma_start(
                            out_e[128 * m : 128 * (m + 1), c0:c1], stg[:]
                        )

                emit_head_backend(0, emit_vx(0), h0_expTs)
                for l in range(1, HEADS_PER_CORE):
                    vx_l = emit_vx(l)
                    f = emit_head_frontend(l)
                    emit_head_backend(l, vx_l, f)
                    if l == 4:
                        # out-projection for heads 0-3's columns rides the
                        # ctxps psum slot between heads 4 and 5, hiding under
                        # head 5's ACT-bound window
                        emit_outproj_cols(0, 512, ctxps_p, "ctxps")

                ctx_es.close()
                with tc.tile_pool(name="ops", bufs=2, space="PSUM") as ops_p:
                    emit_outproj_cols(512, 768, ops_p, "ops")

    nc.finalize()
    return nc


def _get_nc():
    if "nc" not in _NC_CACHE:
        _NC_CACHE["nc"] = _build()
    return _NC_CACHE["nc"]


def kernel(inputs, W_qkv, b_qkv, W_out, b_out, _trace=False, _trace_kwargs=None):
    bf = ml_dtypes.bfloat16
    x = np.asarray(inputs, dtype=np.float32)
    Wq = np.asarray(W_qkv, dtype=np.float32)
    bq = np.asarray(b_qkv, dtype=np.float32)
    Wo = np.asarray(W_out, dtype=np.float32)
    bo = np.asarray(b_out, dtype=np.float32)

    wq_s = np.ascontiguousarray(Wq.reshape(8, P, 3072).transpose(1, 0, 2)).astype(bf)
    wo_s = np.ascontiguousarray(Wo.reshape(16, 64, 1024).transpose(1, 0, 2)).astype(bf)
    bq_s = np.ascontiguousarray(np.broadcast_to(bq[None, :], (P, 3072))).astype(
        np.float32
    )
    bo_s = np.ascontiguousarray(bo.reshape(8, P).T).astype(np.float32)

    in_maps = []
    for c in range(N_CORES):
        xc = x[:, ROWS * c : ROWS * (c + 1), :]  # [3, 256, 1024]
        xt = (
            xc.transpose(2, 0, 1)
            .reshape(1024, 768)
            .reshape(8, P, 768)
            .transpose(1, 0, 2)
        )
        in_maps.append(
            {
                "xt": np.ascontiguousarray(xt).astype(bf),
                "wq": wq_s,
                "bq": bq_s,
                "wo": wo_s,
                "bo": bo_s,
            }
        )

    nc = _get_nc()
    kw = {}
    if _trace:
        kw["trace"] = True
        if _trace_kwargs:
            kw.update(_trace_kwargs)
    res = run_bass_kernel_spmd(nc, in_maps, core_ids=list(range(N_CORES)), **kw)
    outs = res.results

    out = np.empty((6144, 1024), dtype=np.float32)
    for c in range(N_CORES):
        out[768 * c : 768 * (c + 1), :] = np.asarray(
            outs[c]["outt"], dtype=np.float32
        ).T
    if _trace:
        kernel.last_result = res
    return out.reshape(3, SEQ, H)



# revision 22
# speedup vs baseline: 1.1982x; 1.1982x over previous
"""Trainium2 Bass kernel for nn_Attention_82403242541756.

Reference semantics (with the dim-0 chunk bug):
  qkv = inputs @ W_qkv + b_qkv                  # [3, 2048, 3072]
  q, k, v = split(qkv, 3, axis=0)               # batch split! q=batch0, k=batch1, v=batch2
  each chunk [1, 2048, 3072] flat-reinterpreted to (3, 16, 2048, 64) = 48 "heads"
  scoresT softmax (no max needed; |scores| < 2.2), ctx, flat-reinterpret, @ W_out + b_out

Sharding (zero communication): core c takes seq rows [256c, 256c+256) of all 3
batch items. Head g's flat chunk [g*131072, (g+1)*131072) of a batch's [2048*3072]
QKV output aligns exactly with rows [256c, 256c+256) for g in [6c, 6c+6), and the
output-side reinterpret puts head g at rows [128g, 128g+128) of the flattened
[6144, 1024] context, i.e. rows [768c, 768c+768) of the final output per core.

v3 layout/schedule notes:
  - ctx matmul is oriented [s-partitions, d-free] (lhsT = exp chunk, rhs = v):
    ap per matmul is 65 instead of 512, halving ctx PE time, and the softmax
    denominator lands in a per-partition column (no broadcast dance).
  - ctx psum partitions are parity-interleaved (p = 64*(s%2) + s//2 within a
    128-chunk) so that two PE half-transposes produce a [(t%2, d), s'] layout,
    giving the out-projection a full 128-deep contraction (8 steps instead of
    16 -> halves out-proj PE time).
  - PSUM: "sc" ring (3 x 2 banks) carries scores and out-proj psums; "ps" ring
    (2 x 1 bank) carries QKV psums and ctx chunks (with a bf16-bitcast scratch
    region for the half-transposes). 8 banks total, constant the whole kernel.
  - engines execute in-order, so emission is software-pipelined: heads 0-1's
    scores/exps interleave with the QKV units (b2 first so v and ctx(0) are
    ready early); round lf = frontend(lf) lockstep + ctx(lf-1) + outproj(lf-2);
    head 5 runs hh-major so its first 8 ctx chunks fit in round 5, shrinking
    the tail.
"""

import sys

sys.path.insert(0, "/opt/trn_rl_repo")

import numpy as np
import ml_dtypes

from concourse import bacc, bass, mybir, tile, masks
from concourse.bass_utils import run_bass_kernel_spmd

BF16 = mybir.dt.bfloat16
F32 = mybir.dt.float32
AF = mybir.ActivationFunctionType
ALU = mybir.AluOpType

P = 128
N_CORES = 8
SEQ = 2048
H = 1024
HEADS_PER_CORE = 6
ROWS = 256  # seq rows per core
SCALE = float(H) ** -0.5  # 1/32, folded into the exp activation

_NC_CACHE = {}


def _build():
    nc = bacc.Bacc()

    xt_e = nc.declare_dram_parameter("xt", [P, 8, 768], BF16, isOutput=False)
    wq_e = nc.declare_dram_parameter("wq", [P, 8, 3072], BF16, isOutput=False)
    bq_e = nc.declare_dram_parameter("bq", [P, 3072], F32, isOutput=False)
    wo_e = nc.declare_dram_parameter("wo", [P, 8, 1024], BF16, isOutput=False)
    bo_e = nc.declare_dram_parameter("bo", [P, 8], F32, isOutput=False)
    out_e = nc.declare_dram_parameter("outt", [1024, 768], F32, isOutput=True)

    with tile.TileContext(nc) as tc:
        with (
            tc.tile_pool(name="dram", bufs=1, space="DRAM") as dp,
            tc.tile_pool(name="qk", bufs=4) as qkp,
            tc.tile_pool(name="vex", bufs=2) as vxp,
            tc.tile_pool(name="scps", bufs=3, space="PSUM") as scps_p,
            tc.tile_pool(name="psp", bufs=2, space="PSUM") as psp,
            tc.tile_pool(name="expp", bufs=3) as expp,
            tc.tile_pool(name="rs", bufs=2) as rsp,
            tc.tile_pool(name="stg", bufs=3) as stgp,
        ):
            # Padded to 128 cols so the bf16 XBAR DMA-transpose readback is legal.
            yq = dp.tile([12288, 128], BF16)
            yk = dp.tile([12288, 128], BF16)
            yv = dp.tile([12288, 64], BF16)
            yq_v = yq.rearrange("(r j) d -> r j d", j=48)
            yk_v = yk.rearrange("(r j) d -> r j d", j=48)
            yv_v = yv.rearrange("(r j) d -> r (j d)", j=48)

            import contextlib

            es = contextlib.ExitStack()
            w1p = es.enter_context(tc.tile_pool(name="w1", bufs=1))
            ybp = es.enter_context(tc.tile_pool(name="yb", bufs=4))

            # xt first on SP (PE's first matmul gates on it); wq alternates
            # SP/ACT queues; bq rides the idle Pool queue so the ybuf writes
            # aren't stuck behind it on SP
            xt_sb = w1p.tile([P, 8, 768], BF16)
            nc.sync.dma_start(xt_sb[:], xt_e[:])
            wq_sb = w1p.tile([P, 8, 3072], BF16)
            for half in range(2):
                for k in range(8):
                    eng = nc.sync if k % 2 == 0 else nc.scalar
                    eng.dma_start(
                        wq_sb[:, k, 1536 * half : 1536 * (half + 1)],
                        wq_e[:, k, 1536 * half : 1536 * (half + 1)],
                    )
            bq_sb = w1p.tile([P, 3072], F32)
            nc.gpsimd.dma_start(bq_sb[:], bq_e[:])

            def emit_qkv_unit(b, m, nb):
                ps = psp.tile([P, 512], F32, name=f"yps{b}_{m}_{nb}", tag="ps")
                for k in range(8):
                    lhs = xt_sb[:, k, b * 256 + 128 * m : b * 256 + 128 * (m + 1)]
                    nc.tensor.matmul(
                        ps[:],
                        lhsT=lhs,
                        rhs=wq_sb[:, k, 512 * nb : 512 * (nb + 1)],
                        start=(k == 0),
                        stop=(k == 7),
                    )
                if b < 2:
                    # widened [*, *, 128] with zeroed pad cols 64:128 so the
                    # DMA-transpose readback sees defined data
                    ybuf = ybp.tile([P, 8, 128], BF16, tag="ybw")
                    nc.vector.memset(ybuf[:, :, 64:128], 0.0)
                    nc.vector.tensor_tensor(
                        ybuf[:, :, 0:64],
                        ps.rearrange("p (j d) -> p j d", d=64),
                        bq_sb[:, 512 * nb : 512 * (nb + 1)].rearrange(
                            "p (j d) -> p j d", d=64
                        ),
                        ALU.add,
                    )
                    dst = (yq_v if b == 0 else yk_v)[
                        128 * m : 128 * (m + 1), 8 * nb : 8 * (nb + 1), :
                    ]
                    nc.sync.dma_start(dst, ybuf[:])
                else:
                    ybuf = ybp.tile([P, 512], BF16, tag="ybn")
                    nc.vector.tensor_tensor(
                        ybuf[:],
                        ps[:],
                        bq_sb[:, 512 * nb : 512 * (nb + 1)],
                        ALU.add,
                    )
                    nc.sync.dma_start(
                        yv_v[128 * m : 128 * (m + 1), 512 * nb : 512 * (nb + 1)],
                        ybuf[:],
                    )

            def emit_vx(l):
                # vx must ride the SAME queue (SP) as the yv writes: DMA->DMA
                # ordering across queues proved racy on HW (heads whose vx
                # loads land close to the b2 writes came out corrupted)
                vx = vxp.tile([P, 16, 65], BF16, name=f"vx{l}", tag="vx")
                nc.vector.memset(vx[:, :, 64:65], 1.0)
                nc.sync.dma_start(
                    vx[:, :, 0:64],
                    yv[SEQ * l : SEQ * (l + 1), :].rearrange("(so p) d -> p so d", p=P),
                )
                return vx

            def emit_qkT(l):
                # SAME queue (SP) as the yq/yk writes - cross-queue DMA->DMA
                # ordering is racy on HW (see vx note)
                dma_eng = nc.sync
                qT = qkp.tile([P, SEQ], BF16, tag="qk", name=f"qT{l}")
                dma_eng.dma_start(qT[:], yq[SEQ * l : SEQ * (l + 1), :], transpose=True)
                kT = qkp.tile([P, SEQ], BF16, tag="qk", name=f"kT{l}")
                dma_eng.dma_start(kT[:], yk[SEQ * l : SEQ * (l + 1), :], transpose=True)
                return qT, kT

            fe = {}  # head -> (qT, kT, expTs)

            def emit_frontend_alloc(l):
                qT, kT = emit_qkT(l)
                expTs = [
                    expp.tile([P, 8, SEQ], BF16, tag="expT", name=f"expT{l}_{th}")
                    for th in range(2)
                ]
                fe[l] = (qT, kT, expTs)

            def emit_score_exp(l, tt, hh):
                qT, kT, expTs = fe[l]
                th, t8 = tt // 8, tt % 8
                sc = scps_p.tile([P, 1024], F32, name=f"sc{l}_{tt}_{hh}", tag="sc")
                for s2 in range(2):
                    s0 = 1024 * hh + 512 * s2
                    nc.tensor.matmul(
                        sc[:, 512 * s2 : 512 * (s2 + 1)],
                        lhsT=kT[0:64, 128 * tt : 128 * (tt + 1)],
                        rhs=qT[0:64, s0 : s0 + 512],
                        start=True,
                        stop=True,
                    )
                # out AP parity-interleaves each 128-col block (col = 64*(s%2)
                # + (s%128)//2) so ctx lhsT can be a contiguous 1-free-dim
                # slice (HW matmul requires that for the stationary operand)
                nc.scalar.activation(
                    expTs[th][:, t8, 1024 * hh : 1024 * (hh + 1)].rearrange(
                        "p (sb t j) -> p sb j t", t=2, j=64
                    ),
                    sc[:],
                    AF.Exp,
                    scale=SCALE,
                )

            def unit(l, i):
                if l == 5:  # hh-major: first 8 ctx chunks ready mid-round
                    return (i % 16, i // 16)
                return (i // 2, i % 2)

            # ---------------- backend ----------------
            bk = {}  # head -> vx
            stage_all = {}  # head -> [128 (t%2,s//2), 16 sc, 64 d] normalized ctx

            def emit_ctx_chunk(l, scb):
                vx = bk[l]
                _, _, expTs = fe[l]
                if l not in stage_all:
                    stage_all[l] = stgp.tile(
                        [P, 16, 64], BF16, name=f"stga{l}", tag="stga"
                    )
                ctxps = psp.tile([P, 512], F32, name=f"ctxps{l}_{scb}", tag="ps")
                for tt in range(16):
                    th, t8 = tt // 8, tt % 8
                    # cols are already (t%2, s//2)-interleaved by the exp
                    # activation's scatter AP
                    lhsT = expTs[th][:, t8, 128 * scb : 128 * (scb + 1)]
                    nc.tensor.matmul(
                        ctxps[:, 0:65],
                        lhsT=lhsT,
                        rhs=vx[:, tt, :],
                        start=(tt == 0),
                        stop=(tt == 15),
                    )
                rr = rsp.tile([P, 1], F32, tag="rr")
                nc.vector.reciprocal(rr[:], ctxps[:, 64:65])
                nc.vector.tensor_scalar(
                    stage_all[l][:, scb, :], ctxps[:, 0:64], rr[:], None, ALU.mult
                )

            def emit_ctx_gather(l):
                # partition-shift the two parity halves into DRAM rows
                # (sc, j) x cols (t%2, d), then XBAR-transpose straight into
                # the 128-deep-contraction ctxn2 layout
                sa = stage_all[l]
                cd = dp.tile([1024, 128], BF16, name=f"ctxd{l}")
                v = cd.rearrange("(sc j) c -> j sc c", j=64)
                nc.sync.dma_start(v[:, :, 0:64], sa[0:64, :, :])
                nc.sync.dma_start(v[:, :, 64:128], sa[64:128, :, :])
                nc.sync.dma_start(
                    ctxn2[:, l].rearrange("p s j -> p (s j)"), cd[:], transpose=True
                )

            def emit_outproj_m(l, m):
                # rides the scores psum ring - no extra banks, keeps ps parity
                rhs_v = ctxn2[:, l].rearrange("p s (jr u) -> p u s jr", u=8)
                ops = scps_p.tile([P, 1024], F32, name=f"op{l}_{m}", tag="sc")
                for u in range(8):
                    nc.tensor.matmul(
                        ops[:, 0:128],
                        lhsT=wo_sb[:, u, 128 * m : 128 * (m + 1)],
                        rhs=rhs_v[:, u],
                        start=(u == 0),
                        stop=(u == 7),
                    )
                ost = ost_tiles[l]
                nc.vector.tensor_scalar(
                    ost[:, m, :], ops[:, 0:128], bo_sb[:, m : m + 1], None, ALU.add
                )
                if m == 7:
                    nc.sync.dma_start(
                        out_e.rearrange("(m p) r -> p m r", p=P)[
                            :, :, 128 * l : 128 * (l + 1)
                        ],
                        ost[:],
                    )

            # ---------------- emission schedule ----------------
            # prefix: m0 blocks of b0/b1 (covers q/k of heads 0-2)
            for b, m in ((0, 0), (1, 0)):
                for nb in range(6):
                    emit_qkv_unit(b, m, nb)
            emit_frontend_alloc(0)
            emit_frontend_alloc(1)
            # interleave remaining QKV (b2 first -> v/ctx(0) early) with
            # heads 0-1 score units (2 per QKV unit)
            qkv_rest = [
                (b, m, nb) for b, m in ((2, 0), (2, 1), (0, 1), (1, 1))
                for nb in range(6)
            ]
            si = 0
            for qi, (b, m, nb) in enumerate(qkv_rest):
                emit_qkv_unit(b, m, nb)
                for _ in range(2):
                    l, i = divmod(si, 32)
                    emit_score_exp(l, *unit(l, i))
                    si += 1
                if (b, m, nb) == (2, 1, 5):
                    bk[0] = emit_vx(0)
            es.close()  # release w1/yb SBUF for the attention phase

            with (
                tc.tile_pool(name="w2", bufs=1) as w2p,
                tc.tile_pool(name="osb", bufs=2) as osbp,
            ):
                wo_sb = w2p.tile([P, 8, 1024], BF16)
                nc.sync.dma_start(wo_sb[:], wo_e[:])
                bo_sb = w2p.tile([P, 8], F32)
                nc.sync.dma_start(bo_sb[:], bo_e[:])
                # merged transposed-context, 128-deep-contraction layout:
                # ctxn2[p = 64*(t%2) + d, l, sc, j'] with s = 128*sc + 2*j' + t%2
                ctxn2 = w2p.tile([P, HEADS_PER_CORE, 16, 64], BF16)
                ost_tiles = {}

                # phase-1 coda: ctx(0) runs compactly (ACT still owes the
                # last ~8us of head-0/1 exps, covering it), then head-1's
                # remaining units lockstep with outproj(0) riding along.
                bk[1] = emit_vx(1)
                emit_frontend_alloc(2)
                for c in range(16):
                    emit_ctx_chunk(0, c)
                emit_ctx_gather(0)
                ost_tiles[0] = osbp.tile([P, 8, 128], F32, name="ost0", tag="ost")
                for j in range(16):
                    l, i = divmod(si, 32)
                    emit_score_exp(l, *unit(l, i))
                    si += 1
                    if j >= 8:
                        emit_outproj_m(0, j - 8)
                assert si == 64

                # steady rounds: frontend(lf) + ctx(lf-1) + outproj(lf-2)
                for lf in range(2, HEADS_PER_CORE):
                    bk[lf] = emit_vx(lf)
                    if lf + 1 < HEADS_PER_CORE:
                        emit_frontend_alloc(lf + 1)  # prefetch qkT + expT slots
                    lo = lf - 2
                    if lo >= 1:  # op(0) already ran in the coda
                        ost_tiles[lo] = osbp.tile(
                            [P, 8, 128], F32, name=f"ost{lo}", tag="ost"
                        )
                    for i in range(32):
                        emit_score_exp(lf, *unit(lf, i))
                        if lf < 5:
                            if i % 2 == 0:
                                emit_ctx_chunk(lf - 1, i // 2)
                            if lo >= 1 and i % 4 == 1:
                                emit_outproj_m(lo, i // 4)

                        else:
                            # round 5 is hh-major, so th1 exps begin at unit 8
                            # and their expT-slot WAR needs ctx(4) chunks done
                            # at 1/iteration pace; op(3) + ctx(5, 0..7) ride
                            # the lighter second half
                            if i < 16:
                                emit_ctx_chunk(4, i)
                            else:
                                if i == 16:
                                    emit_ctx_gather(4)
                                if i % 2 == 0:
                                    emit_outproj_m(lo, (i - 16) // 2)
                                elif i >= 17:
                                    emit_ctx_chunk(5, (i - 17) // 2)
                    if lf < 5:
                        emit_ctx_gather(lf - 1)

                # tail: ctx(5, 8..15) + outproj(4), then gather(5) + outproj(5)
                ost_tiles[4] = osbp.tile([P, 8, 128], F32, name="ost4", tag="ost")
                for c in range(8, 16):
                    emit_ctx_chunk(5, c)
                    emit_outproj_m(4, c - 8)
                emit_ctx_gather(5)
                ost_tiles[5] = osbp.tile([P, 8, 128], F32, name="ost5", tag="ost")
                for m in range(8):
                    emit_outproj_m(5, m)

    nc.finalize()
    return nc


def _get_nc():
    if "nc" not in _NC_CACHE:
        _NC_CACHE["nc"] = _build()
    return _NC_CACHE["nc"]


def kernel(inputs, W_qkv, b_qkv, W_out, b_out, _trace=False, _trace_kwargs=None):
    bf = ml_dtypes.bfloat16
    x = np.asarray(inputs, dtype=np.float32)
    Wq = np.asarray(W_qkv, dtype=np.float32)
    bq = np.asarray(b_qkv, dtype=np.float32)
    Wo = np.asarray(W_out, dtype=np.float32)
    bo = np.asarray(b_out, dtype=np.float32)

    wq_s = np.ascontiguousarray(Wq.reshape(8, P, 3072).transpose(1, 0, 2)).astype(bf)
    # wo[p = 64*tp + d, u, o] = Wo[f = 128*u + 64*tp + d, o]
    wo_s = np.ascontiguousarray(
        Wo.reshape(8, 2, 64, 1024).transpose(1, 2, 0, 3).reshape(P, 8, 1024)
    ).astype(bf)
    bq_s = np.ascontiguousarray(np.broadcast_to(bq[None, :], (P, 3072))).astype(
        np.float32
    )
    bo_s = np.ascontiguousarray(bo.reshape(8, P).T).astype(np.float32)

    in_maps = []
    for c in range(N_CORES):
        xc = x[:, ROWS * c : ROWS * (c + 1), :]  # [3, 256, 1024]
        xt = (
            xc.transpose(2, 0, 1)
            .reshape(1024, 768)
            .reshape(8, P, 768)
            .transpose(1, 0, 2)
        )
        in_maps.append(
            {
                "xt": np.ascontiguousarray(xt).astype(bf),
                "wq": wq_s,
                "bq": bq_s,
                "wo": wo_s,
                "bo": bo_s,
            }
        )

    nc = _get_nc()
    kw = {}
    if _trace:
        kw["trace"] = True
        if _trace_kwargs:
            kw.update(_trace_kwargs)
    res = run_bass_kernel_spmd(nc, in_maps, core_ids=list(range(N_CORES)), **kw)
    outs = res.results

    out = np.empty((6144, 1024), dtype=np.float32)
    for c in range(N_CORES):
        out[768 * c : 768 * (c + 1), :] = np.asarray(
            outs[c]["outt"], dtype=np.float32
        ).T
    if _trace:
        kernel.last_result = res
    return out.reshape(3, SEQ, H)


# revision 53
# speedup vs baseline: 1.2575x; 1.0494x over previous
"""Trainium2 Bass kernel for nn_Attention_82403242541756.

Reference semantics (with the dim-0 chunk bug):
  qkv = inputs @ W_qkv + b_qkv                  # [3, 2048, 3072]
  q, k, v = split(qkv, 3, axis=0)               # batch split! q=batch0, k=batch1, v=batch2
  each chunk [1, 2048, 3072] flat-reinterpreted to (3, 16, 2048, 64) = 48 "heads"
  scoresT softmax (no max needed; |scores| < 2.2), ctx, flat-reinterpret, @ W_out + b_out

Sharding (zero communication): core c takes seq rows [256c, 256c+256) of all 3
batch items. Head g's flat chunk [g*131072, (g+1)*131072) of a batch's [2048*3072]
QKV output aligns exactly with rows [256c, 256c+256) for g in [6c, 6c+6), and the
output-side reinterpret puts head g at rows [128g, 128g+128) of the flattened
[6144, 1024] context, i.e. rows [768c, 768c+768) of the final output per core.

v3 layout/schedule notes:
  - ctx matmul is oriented [s-partitions, d-free] (lhsT = exp chunk, rhs = v):
    ap per matmul is 65 instead of 512, halving ctx PE time, and the softmax
    denominator lands in a per-partition column (no broadcast dance).
  - ctx psum partitions are parity-interleaved (p = 64*(s%2) + s//2 within a
    128-chunk) so that two PE half-transposes produce a [(t%2, d), s'] layout,
    giving the out-projection a full 128-deep contraction (8 steps instead of
    16 -> halves out-proj PE time).
  - PSUM: "sc" ring (3 x 2 banks) carries scores and out-proj psums; "ps" ring
    (2 x 1 bank) carries QKV psums and ctx chunks (with a bf16-bitcast scratch
    region for the half-transposes). 8 banks total, constant the whole kernel.
  - engines execute in-order, so emission is software-pipelined: heads 0-1's
    scores/exps interleave with the QKV units (b2 first so v and ctx(0) are
    ready early); round lf = frontend(lf) lockstep + ctx(lf-1) + outproj(lf-2);
    head 5 runs hh-major so its first 8 ctx chunks fit in round 5, shrinking
    the tail.
"""

import sys

sys.path.insert(0, "/opt/trn_rl_repo")

import numpy as np
import ml_dtypes

from concourse import bacc, bass, mybir, tile, masks
from concourse.bass_utils import run_bass_kernel_spmd

BF16 = mybir.dt.bfloat16
F32 = mybir.dt.float32
AF = mybir.ActivationFunctionType
ALU = mybir.AluOpType

P = 128
N_CORES = 8
SEQ = 2048
H = 1024
HEADS_PER_CORE = 6
ROWS = 256  # seq rows per core
SCALE = float(H) ** -0.5  # 1/32, folded into the exp activation

_NC_CACHE = {}


def _build():
    nc = bacc.Bacc()

    xt_e = nc.declare_dram_parameter("xt", [P, 8, 768], BF16, isOutput=False)
    wq_e = nc.declare_dram_parameter("wq", [P, 8, 3072], BF16, isOutput=False)
    bq_e = nc.declare_dram_parameter("bq", [P, 3072], F32, isOutput=False)
    wo_e = nc.declare_dram_parameter("wo", [P, 8, 1024], BF16, isOutput=False)
    bo_e = nc.declare_dram_parameter("bo", [P, 8], F32, isOutput=False)
    out_e = nc.declare_dram_parameter("outt", [1024, 768], F32, isOutput=True)

    with tile.TileContext(nc) as tc:
        with (
            tc.tile_pool(name="dram", bufs=1, space="DRAM") as dp,
            tc.tile_pool(name="qk", bufs=4) as qkp,
            tc.tile_pool(name="vex", bufs=2) as vxp,
            tc.tile_pool(name="scps", bufs=3, space="PSUM") as scps_p,
            tc.tile_pool(name="psp", bufs=2, space="PSUM") as psp,
            tc.tile_pool(name="expp", bufs=3) as expp,
            tc.tile_pool(name="rs", bufs=2) as rsp,
            tc.tile_pool(name="stg", bufs=3) as stgp,
        ):
            # Padded to 128 cols so the bf16 XBAR DMA-transpose readback is legal.
            yq = dp.tile([12288, 128], BF16)
            yk = dp.tile([12288, 128], BF16)
            yv = dp.tile([12288, 64], BF16)
            yq_v = yq.rearrange("(r j) d -> r j d", j=48)
            yk_v = yk.rearrange("(r j) d -> r j d", j=48)
            yv_v = yv.rearrange("(r j) d -> r (j d)", j=48)

            import contextlib

            es1 = contextlib.ExitStack()
            es2 = contextlib.ExitStack()
            # es2's pools are created FIRST so es1 (closed earlier) pops in
            # proper stack order
            w1b = es2.enter_context(tc.tile_pool(name="w1b", bufs=1, side="right"))
            ybp = es2.enter_context(tc.tile_pool(name="yb", bufs=4, side="right"))
            w1a = es1.enter_context(tc.tile_pool(name="w1a", bufs=1))

            # phase-1 staging is split so the m1-column half (w1b) can stay
            # alive through round 2, where the last 6 QKV units run in PE
            # slack under the ACT-bound exp stream.
            rr3 = [nc.sync, nc.scalar, nc.gpsimd]
            xt_a = w1a.tile([P, 8, 384], BF16)  # m=0 cols of each b
            xt_b = w1b.tile([P, 8, 384], BF16)  # m=1 cols
            xt_v = xt_e.rearrange("p k (b m r) -> p k b m r", b=3, m=2)
            for kk in range(4):
                ks = slice(2 * kk, 2 * (kk + 1))
                rr3[kk % 3].dma_start(
                    xt_a[:, ks, :].rearrange("p k (b r) -> p k b r", b=3),
                    xt_v[:, ks, :, 0, :],
                )
            wq_lo = w1a.tile([P, 8, 1536], BF16)
            wq_hi = w1b.tile([P, 8, 1536], BF16)
            for k in range(8):
                rr3[(k + 1) % 3].dma_start(wq_lo[:, k, :], wq_e[:, k, 0:1536])
            # xt_b (m1 columns) is first consumed ~60us in - load it after
            # the m0-critical wq_lo stream
            for kk in range(4):
                ks = slice(2 * kk, 2 * (kk + 1))
                rr3[(kk + 1) % 3].dma_start(
                    xt_b[:, ks, :].rearrange("p k (b r) -> p k b r", b=3),
                    xt_v[:, ks, :, 1, :],
                )
            bq_lo = w1a.tile([P, 1536], F32)
            bq_hi = w1b.tile([P, 1536], F32)
            for cc in range(3):
                nc.gpsimd.dma_start(
                    bq_lo[:, 512 * cc : 512 * (cc + 1)],
                    bq_e[:, 512 * cc : 512 * (cc + 1)],
                )
                nc.gpsimd.dma_start(
                    bq_hi[:, 512 * cc : 512 * (cc + 1)],
                    bq_e[:, 1536 + 512 * cc : 1536 + 512 * (cc + 1)],
                )
            # second wq half off SP: the ybuf write stream + qT0/kT0
            # transposes are SP's critical path
            for k in range(8):
                eng = nc.scalar if k % 2 == 0 else nc.gpsimd
                eng.dma_start(wq_hi[:, k, :], wq_e[:, k, 1536:3072])
            # one-time zero of the yq/yk XBAR pad cols (sim finiteness; the
            # transposed pad partitions are never read by compute). m0 rows
            # first so qT0/kT0 aren't gated on the rest.
            z64 = w1a.tile([P, 64], BF16)
            nc.vector.memset(z64[:], 0.0)
            zrow = dp.tile([1, 64], BF16)
            nc.gpsimd.dma_start(zrow[:], z64[0:1, :])
            zsrc = zrow[0:1, :]
            for y in (yq, yk):
                nc.gpsimd.dma_start(y[0:6144, 64:128], zsrc.to_broadcast([6144, 64]))
            for y in (yq, yk):
                nc.gpsimd.dma_start(
                    y[6144:12288, 64:128], zsrc.to_broadcast([6144, 64])
                )

            def emit_qkv_unit(b, m, nb):
                ps = psp.tile([P, 512], F32, name=f"yps{b}_{m}_{nb}", tag="ps")
                xt_t = xt_a if m == 0 else xt_b
                wq_t, nb3 = (wq_lo, nb) if nb < 3 else (wq_hi, nb - 3)
                for k in range(8):
                    lhs = xt_t[:, k, 128 * b : 128 * (b + 1)]
                    nc.tensor.matmul(
                        ps[:],
                        lhsT=lhs,
                        rhs=wq_t[:, k, 512 * nb3 : 512 * (nb3 + 1)],
                        start=(k == 0),
                        stop=(k == 7),
                    )
                if b < 2:
                    # data cols only; the 64:128 XBAR pad cols of yq/yk are
                    # never read by compute (qT/kT partitions 64:128 unused),
                    # so they stay unwritten
                    ybuf = ybp.tile([P, 8, 64], BF16, tag="ybw")
                    nc.vector.tensor_tensor(
                        ybuf[:],
                        ps.rearrange("p (j d) -> p j d", d=64),
                        (bq_lo if nb < 3 else bq_hi)[
                            :, 512 * (nb % 3) : 512 * (nb % 3 + 1)
                        ].rearrange("p (j d) -> p j d", d=64),
                        ALU.add,
                    )
                    dst = (yq_v if b == 0 else yk_v)[
                        128 * m : 128 * (m + 1), 8 * nb : 8 * (nb + 1), 0:64
                    ]
                    nc.sync.dma_start(dst, ybuf[:])
                else:
                    ybuf = ybp.tile([P, 512], BF16, tag="ybn")
                    nc.vector.tensor_tensor(
                        ybuf[:],
                        ps[:],
                        (bq_lo if nb < 3 else bq_hi)[
                            :, 512 * (nb % 3) : 512 * (nb % 3 + 1)
                        ],
                        ALU.add,
                    )
                    nc.sync.dma_start(
                        yv_v[128 * m : 128 * (m + 1), 512 * nb : 512 * (nb + 1)],
                        ybuf[:],
                    )

            def emit_vx(l):
                # vx must ride the SAME queue (SP) as the yv writes: DMA->DMA
                # ordering across queues proved racy on HW (heads whose vx
                # loads land close to the b2 writes came out corrupted)
                vx = vxp.tile([P, 16, 65], BF16, name=f"vx{l}", tag="vx")
                nc.vector.memset(vx[:, :, 64:65], 1.0)
                nc.sync.dma_start(
                    vx[:, :, 0:64],
                    yv[SEQ * l : SEQ * (l + 1), :].rearrange("(so p) d -> p so d", p=P),
                )
                return vx

            def emit_qT(l):
                # SAME queue (SP) as the yq/yk writes - cross-queue DMA->DMA
                # ordering is racy on HW (see vx note)
                qT = qkp.tile([P, SEQ], BF16, tag="qk", name=f"qT{l}")
                nc.sync.dma_start(qT[:], yq[SEQ * l : SEQ * (l + 1), :], transpose=True)
                return qT

            def emit_kT(l):
                kT = qkp.tile([P, SEQ], BF16, tag="qk", name=f"kT{l}")
                nc.sync.dma_start(kT[:], yk[SEQ * l : SEQ * (l + 1), :], transpose=True)
                return kT

            def emit_qkT(l):
                return emit_qT(l), emit_kT(l)

            fe = {}  # head -> (qT, kT, expTs)

            def emit_frontend_alloc(l):
                qT, kT = emit_qkT(l)
                expTs = [
                    expp.tile([P, 8, SEQ], BF16, tag="expT", name=f"expT{l}_{th}")
                    for th in range(2)
                ]
                fe[l] = (qT, kT, expTs)

            def emit_score_exp(l, tt, hh):
                qT, kT, expTs = fe[l]
                th, t8 = tt // 8, tt % 8
                sc = scps_p.tile([P, 1024], F32, name=f"sc{l}_{tt}_{hh}", tag="sc")
                for s2 in range(2):
                    s0 = 1024 * hh + 512 * s2
                    nc.tensor.matmul(
                        sc[:, 512 * s2 : 512 * (s2 + 1)],
                        lhsT=kT[0:64, 128 * tt : 128 * (tt + 1)],
                        rhs=qT[0:64, s0 : s0 + 512],
                        start=True,
                        stop=True,
                    )
                # out AP parity-interleaves each 128-col block (col = 64*(s%2)
                # + (s%128)//2) so ctx lhsT can be a contiguous 1-free-dim
                # slice (HW matmul requires that for the stationary operand)
                nc.scalar.activation(
                    expTs[th][:, t8, 1024 * hh : 1024 * (hh + 1)].rearrange(
                        "p (sb t j) -> p sb j t", t=2, j=64
                    ),
                    sc[:],
                    AF.Exp,
                    scale=SCALE,
                )

            def unit(l, i):
                if l == 5:  # hh-major: first 8 ctx chunks ready mid-round
                    return (i % 16, i // 16)
                return (i // 2, i % 2)

            # ---------------- backend ----------------
            bk = {}  # head -> vx
            stage_all = {}  # head -> [128 (t%2,s//2), 16 sc, 64 d] normalized ctx

            def emit_ctx_chunk(l, scb):
                vx = bk[l]
                _, _, expTs = fe[l]
                if l not in stage_all:
                    stage_all[l] = stgp.tile(
                        [P, 16, 64], BF16, name=f"stga{l}", tag="stga"
                    )
                ctxps = psp.tile([P, 512], F32, name=f"ctxps{l}_{scb}", tag="ps")
                for tt in range(16):
                    th, t8 = tt // 8, tt % 8
                    # cols are already (t%2, s//2)-interleaved by the exp
                    # activation's scatter AP
                    lhsT = expTs[th][:, t8, 128 * scb : 128 * (scb + 1)]
                    nc.tensor.matmul(
                        ctxps[:, 0:65],
                        lhsT=lhsT,
                        rhs=vx[:, tt, :],
                        start=(tt == 0),
                        stop=(tt == 15),
                    )
                rr = rsp.tile([P, 1], F32, tag="rr")
                nc.vector.reciprocal(rr[:], ctxps[:, 64:65])
                nc.vector.tensor_scalar(
                    stage_all[l][:, scb, :], ctxps[:, 0:64], rr[:], None, ALU.mult
                )

            def emit_ctx_gather(l, half=None, eng=None):
                eng = eng or nc.sync
                # partition-shift the two parity halves into DRAM rows
                # (sc, j) x cols (t%2, d), then XBAR-transpose straight into
                # the 128-deep-contraction ctxn2 layout
                sa = stage_all[l]
                if l not in ctxd_tiles:
                    ctxd_tiles[l] = dp.tile([1024, 128], BF16, name=f"ctxd{l}")
                cd = ctxd_tiles[l]
                s0, s1 = (0, 16) if half is None else (8 * half, 8 * (half + 1))
                v = cd.rearrange("(sc j) c -> j sc c", j=64)
                eng.dma_start(v[:, s0:s1, 0:64], sa[0:64, s0:s1, :])
                eng.dma_start(v[:, s0:s1, 64:128], sa[64:128, s0:s1, :])
                dst = (
                    ctxn5b[:, :, :]
                    if (l == 5 and half == 1)
                    else ctxn2[:, l, s0:s1, :]
                )
                eng.dma_start(
                    dst.rearrange("p s j -> p (s j)"),
                    cd[64 * s0 : 64 * s1, :],
                    transpose=True,
                )

            def emit_outproj_m(l, m, half=None, out_eng=None):
                # rides the scores psum ring - no extra banks, keeps ps parity.
                # half splits output rows by sc-half (r < 64 needs only ctxn2
                # sc 0..8), letting the last head's first half run before its
                # final ctx chunks are gathered.
                if l == 5 and half == 1:
                    rhs_v = ctxn5b.rearrange("p s (jr u) -> p u s jr", u=8)
                    rv_off = 8
                else:
                    rhs_v = ctxn2[:, l].rearrange("p s (jr u) -> p u s jr", u=8)
                    rv_off = 0
                r0, r1 = (0, 128) if half is None else (64 * half, 64 * (half + 1))
                n = r1 - r0
                ops = scps_p.tile([P, 1024], F32, name=f"op{l}_{m}_{r0}", tag="sc")
                for u in range(8):
                    nc.tensor.matmul(
                        ops[:, 0:n],
                        lhsT=wo_sb[:, u, 128 * m : 128 * (m + 1)],
                        rhs=rhs_v[:, u, r0 // 8 - rv_off : r1 // 8 - rv_off, :],
                        start=(u == 0),
                        stop=(u == 7),
                    )
                ost = ost_tiles[l]
                nc.vector.tensor_scalar(
                    ost[:, m, r0:r1], ops[:, 0:n], bo_sb[:, m : m + 1], None, ALU.add
                )
                if m == 3 and l == 5 and half == 1:
                    # early half of the very last output DMA
                    nc.sync.dma_start(
                        out_e.rearrange("(m p) r -> p m r", p=P)[
                            :, 0:4, 128 * l + r0 : 128 * l + r1
                        ],
                        ost[:, 0:4, r0:r1],
                    )
                if m == 7:
                    ms = 4 if (l == 5 and half == 1) else 0
                    (out_eng or nc.sync).dma_start(
                        out_e.rearrange("(m p) r -> p m r", p=P)[
                            :, ms:8, 128 * l + r0 : 128 * l + r1
                        ],
                        ost[:, ms:8, r0:r1],
                    )

            # ---------------- emission schedule ----------------
            # prefix: m0 blocks of b0/b1 (covers q/k of heads 0-2)
            for nb in range(6):
                emit_qkv_unit(0, 0, nb)
            qT0 = emit_qT(0)
            for nb in range(6):
                emit_qkv_unit(1, 0, nb)
            kT0 = emit_kT(0)
            expTs0 = [
                expp.tile([P, 8, SEQ], BF16, tag="expT", name=f"expT0_{th}")
                for th in range(2)
            ]
            fe[0] = (qT0, kT0, expTs0)
            emit_frontend_alloc(1)
            # interleave remaining QKV (b2 first -> v/ctx(0) early) with
            # heads 0-1 score units (2 per QKV unit)
            qkv_rest = [(2, m, nb) for m in range(2) for nb in range(6)] + [
                (b, 1, nb) for b in range(2) for nb in range(3)
            ]
            si = 0
            for qi, (b, m, nb) in enumerate(qkv_rest):
                emit_qkv_unit(b, m, nb)
                for _ in range(2 if qi % 3 == 0 else 3):
                    l, i = divmod(si, 32)
                    emit_score_exp(l, *unit(l, i))
                    si += 1
                if (b, m, nb) == (2, 1, 5):
                    bk[0] = emit_vx(0)
            es1.close()  # release the m0-half staging

            with (
                tc.tile_pool(name="w2", bufs=1) as w2p,
                tc.tile_pool(name="osb", bufs=2) as osbp,
            ):
                wo_sb = w2p.tile([P, 8, 1024], BF16)
                nc.sync.dma_start(wo_sb[:], wo_e[:])
                bo_sb = w2p.tile([P, 8], F32)
                nc.sync.dma_start(bo_sb[:], bo_e[:])
                # merged transposed-context, 128-deep-contraction layout:
                # ctxn2[p = 64*(t%2) + d, l, sc, j'] with s = 128*sc + 2*j' + t%2
                ctxn2 = w2p.tile([P, HEADS_PER_CORE, 16, 64], BF16)
                # head 5's sc 8..16 half lives in its own tile so the tail
                # gather's transpose doesn't false-WAR against op5A's reads
                ctxn5b = w2p.tile([P, 8, 64], BF16)
                ost_tiles = {}
                ctxd_tiles = {}

                # phase-1 coda: ctx(0) runs compactly (ACT still owes the
                # last ~8us of head-0/1 exps, covering it), then head-1's
                # remaining units lockstep with outproj(0) riding along.
                bk[1] = emit_vx(1)
                emit_frontend_alloc(2)
                for c in range(16):
                    emit_ctx_chunk(0, c)
                emit_ctx_gather(0)
                ost_tiles[0] = osbp.tile([P, 8, 128], F32, name="ost0", tag="ost")
                for j in range(16):
                    l, i = divmod(si, 32)
                    emit_score_exp(l, *unit(l, i))
                    si += 1
                    if j == 2:
                        emit_qkv_unit(0, 1, 3)
                    if j == 5:
                        emit_qkv_unit(1, 1, 3)
                    if j == 8:
                        emit_qkv_unit(0, 1, 4)
                    if j == 11:
                        emit_qkv_unit(0, 1, 5)
                    if j >= 8:
                        emit_outproj_m(0, j - 8)
                assert si == 64

                # steady rounds: frontend(lf) + ctx(lf-1) + outproj(lf-2)
                qkv_round2 = [(1, 1, 4), (1, 1, 5)]
                for lf in range(2, HEADS_PER_CORE):
                    bk[lf] = emit_vx(lf)
                    lo = lf - 2
                    if lo >= 1:  # op(0) already ran in the coda
                        ost_tiles[lo] = osbp.tile(
                            [P, 8, 128], F32, name=f"ost{lo}", tag="ost"
                        )
                    for i in range(32):
                        emit_score_exp(lf, *unit(lf, i))
                        if lf < 5:
                            if i % 2 == 0:
                                emit_ctx_chunk(lf - 1, i // 2)
                            if lf == 2 and i % 16 == 1:
                                emit_qkv_unit(*qkv_round2[i // 16])
                            if lo >= 1 and i % 4 == 1:
                                emit_outproj_m(lo, i // 4)

                        else:
                            # round 5 is hh-major, so th1 exps begin at unit 8
                            # and their expT-slot WAR needs ctx(4) chunks done
                            # at 1/iteration pace; op(3) + ctx(5, 0..7) ride
                            # the lighter second half
                            if i < 16:
                                emit_ctx_chunk(4, i)
                            else:
                                if i == 16:
                                    emit_ctx_gather(4)
                                if i % 2 == 0:
                                    emit_outproj_m(lo, (i - 16) // 2)
                                elif i >= 17:
                                    emit_ctx_chunk(5, (i - 17) // 2)
                    if lf < 5:
                        emit_ctx_gather(lf - 1)
                    else:
                        emit_ctx_gather(5, half=0)
                    if lf + 1 < HEADS_PER_CORE:
                        # prefetch at round END: head lf+1's qkT needs the m1
                        # rows, whose last QKV units run inside round 2
                        emit_frontend_alloc(lf + 1)
                    if lf == 2:
                        es2.close()  # QKV fully done; release the m1 staging

                # tail: ctx(5, 8..15) interleaved with outproj(5) first-half
                # (needs only the sc 0..7 gather done at round-5 end) and
                # outproj(4); then the second-half gather and outproj(5B)
                ost_tiles[4] = osbp.tile([P, 8, 128], F32, name="ost4", tag="ost")
                ost_tiles[5] = osbp.tile([P, 8, 128], F32, name="ost5", tag="ost")
                for c in range(8, 16):
                    emit_ctx_chunk(5, c)
                    # outt-A on the post-exp-idle ACT queue so SP's gather
                    # transpose isn't queue-blocked behind it
                    emit_outproj_m(5, c - 8, half=0, out_eng=nc.scalar)
                emit_ctx_gather(5, half=1)
                for m in range(8):
                    emit_outproj_m(4, m)
                for m in range(8):
                    emit_outproj_m(5, m, half=1)

    nc.finalize()
    return nc


def _get_nc():
    if "nc" not in _NC_CACHE:
        _NC_CACHE["nc"] = _build()
    return _NC_CACHE["nc"]


def kernel(inputs, W_qkv, b_qkv, W_out, b_out, _trace=False, _trace_kwargs=None):
    bf = ml_dtypes.bfloat16
    x = np.asarray(inputs, dtype=np.float32)
    Wq = np.asarray(W_qkv, dtype=np.float32)
    bq = np.asarray(b_qkv, dtype=np.float32)
    Wo = np.asarray(W_out, dtype=np.float32)
    bo = np.asarray(b_out, dtype=np.float32)

    wq_s = np.ascontiguousarray(Wq.reshape(8, P, 3072).transpose(1, 0, 2)).astype(bf)
    # wo[p = 64*tp + d, u, o] = Wo[f = 128*u + 64*tp + d, o]
    wo_s = np.ascontiguousarray(
        Wo.reshape(8, 2, 64, 1024).transpose(1, 2, 0, 3).reshape(P, 8, 1024)
    ).astype(bf)
    bq_s = np.ascontiguousarray(np.broadcast_to(bq[None, :], (P, 3072))).astype(
        np.float32
    )
    bo_s = np.ascontiguousarray(bo.reshape(8, P).T).astype(np.float32)

    in_maps = []
    for c in range(N_CORES):
        xc = x[:, ROWS * c : ROWS * (c + 1), :]  # [3, 256, 1024]
        xt = (
            xc.transpose(2, 0, 1)
            .reshape(1024, 768)
            .reshape(8, P, 768)
            .transpose(1, 0, 2)
        )
        in_maps.append(
            {
                "xt": np.ascontiguousarray(xt).astype(bf),
                "wq": wq_s,
                "bq": bq_s,
                "wo": wo_s,
                "bo": bo_s,
            }
        )

    nc = _get_nc()
    kw = {}
    if _trace:
        kw["trace"] = True
        if _trace_kwargs:
            kw.update(_trace_kwargs)
    res = run_bass_kernel_spmd(nc, in_maps, core_ids=list(range(N_CORES)), **kw)
    outs = res.results

    out = np.empty((6144, 1024), dtype=np.float32)
    for c in range(N_CORES):
        out[768 * c : 768 * (c + 1), :] = np.asarray(
            outs[c]["outt"], dtype=np.float32
        ).T
    if _trace:
        kernel.last_result = res
    return out.reshape(3, SEQ, H)
